# revision 8
# baseline (speedup 1.0000x reference)
"""Additive (Bahdanau) attention on 8 TRN2 NeuronCores.

Problem shapes (hardcoded): B=4, n=512, m=1024, dq=dk=dv=256, h=128.
Sharding: data-parallel over (batch, n-half) -> 8 independent shards, one per
core, no collectives. Each core computes 256 query rows against its batch's
1024 keys/values.

Per-core algorithm (layouts are partition-major on h=128):
  tqT[h, i] = Wq.T @ qT + (bq+bk)      (PE, bf16 in / f32 accum)
  tkT[h, j] = Wk.T @ kT                (PE, bf16 in / f32 accum)
  for each block of ib query rows:
    S[h, il, j] = tkT[h, j] + tqT[h, i]    (DVE tensor_scalar, f32)
    T = tanh(S)                            (ACT, one instr per block, bf16 out)
    score[i, j] += wv . T[:, il, j]  via accumulating matmuls whose lhsT is a
        sliding window of a (h, 256) matrix holding wv in column 128, so PSUM
        row i receives wv.T @ T_i          (PE, bf16)
  per 128-row group: mask-add, exp (+row-sum via accum_out), transpose the
  bf16 weights, weight @ values, scale by 1/rowsum, DMA out.

q/k/Wq/Wk/v are shipped as bf16 (bit-identical to casting on device — they
only feed bf16 matmuls) so no cast sits on the startup critical path.

The ScalarE tanh stream is the roofline: 256*1024*128 / 128 lanes / 1.2 GHz
= 218 us busy per core; everything else hides underneath it.  Block sizes
ramp small at the start (first tanh as early as possible) and at the end
(shortest tail after the last tanh).
"""

import numpy as np
import ml_dtypes

import concourse.bass as bass
import concourse.mybir as mybir
import concourse.tile as tile
from concourse import bacc
from concourse.bass_utils import run_bass_kernel_spmd
from concourse.masks import make_identity

F32 = mybir.dt.float32
BF16 = mybir.dt.bfloat16

B, N, M = 4, 512, 1024
DQ, DK, DV, H = 256, 256, 256, 128
N_CORES = 8
N_LOC = B * N // N_CORES  # 256 query rows per core
IB = 8                    # steady-state query rows per tanh block
NEG = -40.0               # additive mask value (exp(score-40) == 0 relative to valid)


def _blocks(first, last):
    ramp_up = [2, 2, 4] if first else []
    ramp_dn = [4, 2, 2] if last else []
    steady = (128 - sum(ramp_up) - sum(ramp_dn)) // IB
    return ramp_up + [IB] * steady + ramp_dn


def build_nc():
    nc = bacc.Bacc("TRN2", target_bir_lowering=False)

    qT_d = nc.declare_dram_parameter("qT", [DQ, N_LOC], BF16, isOutput=False)
    kT_d = nc.declare_dram_parameter("kT", [DK, M], BF16, isOutput=False)
    v_d = nc.declare_dram_parameter("v", [M, DV], BF16, isOutput=False)
    badd_d = nc.declare_dram_parameter("badd", [N_LOC, M], F32, isOutput=False)
    wq_d = nc.declare_dram_parameter("Wq", [DQ, H], BF16, isOutput=False)
    wk_d = nc.declare_dram_parameter("Wk", [DK, H], BF16, isOutput=False)
    wv_d = nc.declare_dram_parameter("wv", [H, 1], F32, isOutput=False)
    bqk_d = nc.declare_dram_parameter("bqk", [H, 1], F32, isOutput=False)
    out_d = nc.declare_dram_parameter("out", [N_LOC, DV], F32, isOutput=True)

    tanh = mybir.ActivationFunctionType.Tanh
    expf = mybir.ActivationFunctionType.Exp

    with tile.TileContext(nc) as tc:
        with tc.tile_pool(name="const", bufs=1) as cpool:
            # ---- persistent tiles ----
            dummy = cpool.tile([H, 1], F32)
            wv_sb = cpool.tile([H, 1], F32)
            bqk_sb = cpool.tile([H, 1], F32)
            msl = cpool.tile([H, 2 * H], BF16)
            ident = cpool.tile([H, H], BF16)
            tqT_sb = cpool.tile([H, N_LOC], F32)
            tkT_sb = cpool.tile([H, M], F32)
            v_bf = cpool.tile([128, M // 128, DV], BF16)
            badd_sb = cpool.tile([128, N_LOC // 128, M], F32)
            wk_bf = cpool.tile([128, 2, H], BF16)
            wq_bf = cpool.tile([128, 2, H], BF16)
            qt_bf = cpool.tile([128, 2, N_LOC], BF16)
            kt_bf = cpool.tile([128, 2, M], BF16)

            # critical-path loads on the sync (HWDGE) queue, keys first
            kt_r = kT_d.rearrange("(t p) j -> p t j", p=128)
            nc.sync.dma_start(kt_bf[:, 0, :], kt_r[:, 0, :])
            nc.gpsimd.dma_start(kt_bf[:, 1, :], kt_r[:, 1, :])
            nc.sync.dma_start(wk_bf[:, :, :], wk_d.rearrange("(t p) h -> p t h", p=128))
            nc.sync.dma_start(qt_bf[:, :, :], qT_d.rearrange("(t p) i -> p t i", p=128))
            nc.sync.dma_start(wq_bf[:, :, :], wq_d.rearrange("(t p) h -> p t h", p=128))
            nc.sync.dma_start(wv_sb[:, :], wv_d[:, :])
            nc.sync.dma_start(bqk_sb[:, :], bqk_d[:, :])

            # warm the ACT tanh/exp table set while DMAs run
            nc.vector.memset(dummy[:, :], 0.0)
            nc.scalar.activation(dummy[:, :], dummy[:, :], tanh)

            with (
                tc.tile_pool(name="setup_psum", bufs=2, space=bass.MemorySpace.PSUM) as spp,
            ):
                def tk_half(jh):
                    tk_ps = spp.tile([H, 512], F32, tag="tkps", name=f"tkps{jh}")
                    for t in range(2):
                        nc.tensor.matmul(
                            tk_ps[:, :], wk_bf[:, t, :],
                            kt_bf[:, t, jh * 512 : (jh + 1) * 512],
                            start=(t == 0), stop=(t == 1),
                        )
                    nc.vector.tensor_copy(tkT_sb[:, jh * 512 : (jh + 1) * 512], tk_ps[:, :])

                tk_half(0)
                tq_ps = spp.tile([H, N_LOC], F32)
                for t in range(2):
                    nc.tensor.matmul(
                        tq_ps[:, :], wq_bf[:, t, :], qt_bf[:, t, :],
                        start=(t == 0), stop=(t == 1),
                    )
                # fold bq+bk into the query transform
                nc.vector.tensor_scalar_add(tqT_sb[:, :], tq_ps[:, :], bqk_sb[:, 0:1])
                tk_half(1)

            # sliding-window lhsT: wv lives in column 128; slice [128-i, 256-i)
            # puts wv in window-column i, zeros elsewhere.
            nc.gpsimd.memset(msl[:, :], 0.0)
            nc.vector.tensor_copy(msl[:, H : H + 1], wv_sb[:, :])
            make_identity(nc, ident[:, :])

            # ---- main pipeline ----
            with (
                tc.tile_pool(name="s_pool", bufs=3) as s_pool,
                tc.tile_pool(name="t_pool", bufs=2) as t_pool,
                tc.tile_pool(name="sm_pool", bufs=2) as sm_pool,
                tc.tile_pool(name="w_pool", bufs=2) as w_pool,
                tc.tile_pool(name="wt_pool", bufs=2) as wt_pool,
                tc.tile_pool(name="o_pool", bufs=2) as o_pool,
                tc.tile_pool(name="stat", bufs=4) as stat,
                tc.tile_pool(name="score_ps", bufs=4, space=bass.MemorySpace.PSUM) as score_pp,
                tc.tile_pool(name="wt_ps", bufs=2, space=bass.MemorySpace.PSUM) as wt_pp,
                tc.tile_pool(name="out_ps", bufs=2, space=bass.MemorySpace.PSUM) as out_pp,
            ):
                n_groups = N_LOC // 128
                for g in range(n_groups):
                    sc = [
                        score_pp.tile([128, 512], F32, tag="sc", name=f"sc{g}_{jh}")
                        for jh in range(2)
                    ]
                    ig = 0
                    for bi, ib in enumerate(_blocks(g == 0, g == n_groups - 1)):
                        S = s_pool.tile([128, IB, M], F32, tag="S", name=f"S{g}_{bi}")
                        if g == 0 and bi == 0:
                            for jh in range(2):
                                js = slice(jh * 512, (jh + 1) * 512)
                                for il in range(ib):
                                    i = g * 128 + ig + il
                                    nc.vector.tensor_scalar_add(
                                        S[:, il, js], tkT_sb[:, js], tqT_sb[:, i : i + 1]
                                    )
                        else:
                            for il in range(ib):
                                i = g * 128 + ig + il
                                nc.vector.tensor_scalar_add(
                                    S[:, il, :], tkT_sb[:, :], tqT_sb[:, i : i + 1]
                                )
                        T = t_pool.tile([128, IB, M], BF16, tag="T", name=f"T{g}_{bi}")
                        nc.scalar.activation(T[:, :ib, :], S[:, :ib, :], tanh)
                        if g == 0 and bi == 0:
                            # non-critical loads, issued once the hot path rolls
                            nc.sync.dma_start(
                                v_bf[:, :, :], v_d.rearrange("(t p) v -> p t v", p=128)
                            )
                            nc.sync.dma_start(
                                badd_sb[:, :, :],
                                badd_d.rearrange("(t p) j -> p t j", p=128),
                            )
                        for il in range(ib):
                            r = ig + il
                            for jh in range(2):
                                nc.tensor.matmul(
                                    sc[jh][:, :],
                                    msl[:, H - r : 2 * H - r],
                                    T[:, il, jh * 512 : (jh + 1) * 512],
                                    start=(r == 0), stop=(r == 127),
                                )
                        ig += ib

                    # ---- softmax + output for this 128-row group ----
                    scm = sm_pool.tile([128, M], F32)
                    for jh in range(2):
                        nc.vector.tensor_add(
                            scm[:, jh * 512 : (jh + 1) * 512],
                            sc[jh][:, :],
                            badd_sb[:, g, jh * 512 : (jh + 1) * 512],
                        )
                    wexp = w_pool.tile([128, M], BF16)
                    rowsum = stat.tile([128, 1], F32)
                    nc.scalar.activation(
                        wexp[:, :], scm[:, :], expf, accum_out=rowsum[:, 0:1]
                    )
                    recip = stat.tile([128, 1], F32)
                    nc.vector.reciprocal(recip[:, 0:1], rowsum[:, 0:1])

                    wt_sb = wt_pool.tile([128, M // 128, 128], BF16)
                    for jt in range(M // 128):
                        wt_ps = wt_pp.tile([128, 128], BF16)
                        nc.tensor.transpose(
                            wt_ps[:, :], wexp[:, jt * 128 : (jt + 1) * 128], ident[:, :]
                        )
                        nc.vector.tensor_copy(wt_sb[:, jt, :], wt_ps[:, :])

                    out_ps = out_pp.tile([128, DV], F32)
                    for jt in range(M // 128):
                        nc.tensor.matmul(
                            out_ps[:, :], wt_sb[:, jt, :], v_bf[:, jt, :],
                            start=(jt == 0), stop=(jt == M // 128 - 1),
                        )
                    out_sb = o_pool.tile([128, DV], F32)
                    nc.vector.tensor_scalar_mul(out_sb[:, :], out_ps[:, :], recip[:, 0:1])
                    nc.sync.dma_start(out_d[g * 128 : (g + 1) * 128, :], out_sb[:, :])

    nc.compile()
    return nc


_NC_CACHE = []


def _get_nc():
    if not _NC_CACHE:
        _NC_CACHE.append(build_nc())
    return _NC_CACHE[0]


def make_in_maps(queries, keys, values, mask, Wq, bq, Wk, bk, wv, bv):
    f32 = np.float32
    bf = ml_dtypes.bfloat16
    badd_full = (mask.astype(f32) - 1.0) * -NEG  # 0 where valid, NEG where masked
    wv_col = np.ascontiguousarray(wv.reshape(H, 1).astype(f32))
    bqk = np.ascontiguousarray((bq + bk).reshape(H, 1).astype(f32))
    wq = np.ascontiguousarray(Wq.astype(bf))
    wk = np.ascontiguousarray(Wk.astype(bf))
    in_maps = []
    for c in range(N_CORES):
        b, half = divmod(c, 2)
        rows = slice(half * N_LOC, (half + 1) * N_LOC)
        in_maps.append(
            {
                "qT": np.ascontiguousarray(queries[b, rows].T.astype(bf)),
                "kT": np.ascontiguousarray(keys[b].T.astype(bf)),
                "v": np.ascontiguousarray(values[b].astype(bf)),
                "badd": np.ascontiguousarray(badd_full[b, rows]),
                "Wq": wq,
                "Wk": wk,
                "wv": wv_col,
                "bqk": bqk,
            }
        )
    return in_maps


def gather_out(results):
    out = np.zeros((B, N, DV), np.float32)
    for c in range(N_CORES):
        b, half = divmod(c, 2)
        out[b, half * N_LOC : (half + 1) * N_LOC] = results[c]["out"]
    return out


def kernel(**inputs):
    nc = _get_nc()
    in_maps = make_in_maps(**inputs)
    res = run_bass_kernel_spmd(nc, in_maps, core_ids=list(range(N_CORES)))
    return gather_out(res.results)


# revision 10
# speedup vs baseline: 1.1953x; 1.1953x over previous
"""Additive (Bahdanau) attention on 8 TRN2 NeuronCores.

Problem shapes (hardcoded): B=4, n=512, m=1024, dq=dk=dv=256, h=128.
Sharding: data-parallel over (batch, n-half) -> 8 independent shards, one per
core, no collectives. Each core computes 256 query rows against its batch's
1024 keys/values.

Per-core algorithm (layouts are partition-major on h=128):
  tqT[h, i] = Wq.T @ qT + (bq+bk)      (PE, bf16 in / f32 accum)
  tkT[h, j] = Wk.T @ kT                (PE, bf16 in / f32 accum)
  for each block of ib query rows:
    S[h, il, j] = tkT[h, j] + tqT[h, i]    (DVE tensor_scalar, f32)
    T = tanh(S)                            (ACT, one instr per block, bf16 out)
    score[i, j] += wv . T[:, il, j]  via accumulating matmuls whose lhsT is a
        sliding window of a (h, 256) matrix holding wv in column 128, so PSUM
        row i receives wv.T @ T_i          (PE, bf16)
  per 128-row group: mask-add, exp (+row-sum via accum_out), transpose the
  bf16 weights, weight @ values, scale by 1/rowsum, DMA out.

q/k/Wq/Wk/v are shipped as bf16 (bit-identical to casting on device — they
only feed bf16 matmuls) so no cast sits on the startup critical path.

The ScalarE tanh stream is the roofline: 256*1024*128 / 128 lanes / 1.2 GHz
= 218 us busy per core; everything else hides underneath it.  Block sizes
ramp small at the start (first tanh as early as possible) and at the end
(shortest tail after the last tanh).
"""

import numpy as np
import ml_dtypes

import concourse.bass as bass
import concourse.mybir as mybir
import concourse.tile as tile
from concourse import bacc
from concourse.bass_utils import run_bass_kernel_spmd
from concourse.masks import make_identity

F32 = mybir.dt.float32
BF16 = mybir.dt.bfloat16

B, N, M = 4, 512, 1024
DQ, DK, DV, H = 256, 256, 256, 128
N_CORES = 8
N_LOC = B * N // N_CORES  # 256 query rows per core
IB = 8                    # steady-state query rows per tanh block
NEG = -40.0               # additive mask value (exp(score-40) == 0 relative to valid)


def _blocks(first, last):
    ramp_up = [2, 2, 4] if first else []
    ramp_dn = [4, 2, 2] if last else []
    steady = (128 - sum(ramp_up) - sum(ramp_dn)) // IB
    return ramp_up + [IB] * steady + ramp_dn


def build_nc():
    nc = bacc.Bacc("TRN2", target_bir_lowering=False)

    qT_d = nc.declare_dram_parameter("qT", [DQ, N_LOC], BF16, isOutput=False)
    kT_d = nc.declare_dram_parameter("kT", [DK, M], BF16, isOutput=False)
    v_d = nc.declare_dram_parameter("v", [M, DV], BF16, isOutput=False)
    badd_d = nc.declare_dram_parameter("badd", [N_LOC, M], F32, isOutput=False)
    wq_d = nc.declare_dram_parameter("Wq", [DQ, H], BF16, isOutput=False)
    wk_d = nc.declare_dram_parameter("Wk", [DK, H], BF16, isOutput=False)
    wv_d = nc.declare_dram_parameter("wv", [H, 1], F32, isOutput=False)
    bqk_d = nc.declare_dram_parameter("bqk", [H, 1], F32, isOutput=False)
    out_d = nc.declare_dram_parameter("out", [N_LOC, DV], F32, isOutput=True)

    tanh = mybir.ActivationFunctionType.Tanh
    expf = mybir.ActivationFunctionType.Exp

    with tile.TileContext(nc) as tc:
        with tc.tile_pool(name="const", bufs=1) as cpool:
            # ---- persistent tiles ----
            dummy = cpool.tile([H, 1], F32)
            wv_sb = cpool.tile([H, 1], F32)
            bqk_sb = cpool.tile([H, 1], F32)
            msl = cpool.tile([H, 2 * H], BF16)
            ident = cpool.tile([H, H], BF16)
            tqT_sb = cpool.tile([H, N_LOC], F32)
            tkT_sb = cpool.tile([H, M], F32)
            v_bf = cpool.tile([128, M // 128, DV], BF16)
            badd_sb = cpool.tile([128, N_LOC // 128, M], F32)
            wk_bf = cpool.tile([128, 2, H], BF16)
            wq_bf = cpool.tile([128, 2, H], BF16)
            qt_bf = cpool.tile([128, 2, N_LOC], BF16)
            kt_bf = cpool.tile([128, 2, M], BF16)

            # critical-path loads on the sync (HWDGE) queue, keys first
            kt_r = kT_d.rearrange("(t p) j -> p t j", p=128)
            nc.sync.dma_start(kt_bf[:, 0, :], kt_r[:, 0, :])
            nc.gpsimd.dma_start(kt_bf[:, 1, :], kt_r[:, 1, :])
            nc.sync.dma_start(wk_bf[:, :, :], wk_d.rearrange("(t p) h -> p t h", p=128))
            nc.sync.dma_start(qt_bf[:, :, :], qT_d.rearrange("(t p) i -> p t i", p=128))
            nc.sync.dma_start(wq_bf[:, :, :], wq_d.rearrange("(t p) h -> p t h", p=128))
            nc.sync.dma_start(wv_sb[:, :], wv_d[:, :])
            nc.sync.dma_start(bqk_sb[:, :], bqk_d[:, :])

            # warm the ACT tanh/exp table set while DMAs run
            nc.vector.memset(dummy[:, :], 0.0)
            nc.scalar.activation(dummy[:, :], dummy[:, :], tanh)

            with (
                tc.tile_pool(name="setup_psum", bufs=2, space=bass.MemorySpace.PSUM) as spp,
            ):
                def tk_half(jh):
                    tk_ps = spp.tile([H, 512], F32, tag="tkps", name=f"tkps{jh}")
                    for t in range(2):
                        nc.tensor.matmul(
                            tk_ps[:, :], wk_bf[:, t, :],
                            kt_bf[:, t, jh * 512 : (jh + 1) * 512],
                            start=(t == 0), stop=(t == 1),
                        )
                    nc.vector.tensor_copy(tkT_sb[:, jh * 512 : (jh + 1) * 512], tk_ps[:, :])

                tk_half(0)
                tq_ps = spp.tile([H, N_LOC], F32)
                for t in range(2):
                    nc.tensor.matmul(
                        tq_ps[:, :], wq_bf[:, t, :], qt_bf[:, t, :],
                        start=(t == 0), stop=(t == 1),
                    )
                # fold bq+bk into the query transform
                nc.vector.tensor_scalar_add(tqT_sb[:, :], tq_ps[:, :], bqk_sb[:, 0:1])
                tk_half(1)

            # sliding-window lhsT: wv lives in column 128; slice [128-i, 256-i)
            # puts wv in window-column i, zeros elsewhere.
            nc.gpsimd.memset(msl[:, :], 0.0)
            nc.vector.tensor_copy(msl[:, H : H + 1], wv_sb[:, :])
            make_identity(nc, ident[:, :])

            # ---- main pipeline ----
            with (
                tc.tile_pool(name="s_pool", bufs=3) as s_pool,
                tc.tile_pool(name="t_pool", bufs=2) as t_pool,
                tc.tile_pool(name="sm_pool", bufs=2) as sm_pool,
                tc.tile_pool(name="w_pool", bufs=2) as w_pool,
                tc.tile_pool(name="wt_pool", bufs=2) as wt_pool,
                tc.tile_pool(name="o_pool", bufs=2) as o_pool,
                tc.tile_pool(name="stat", bufs=4) as stat,
                tc.tile_pool(name="score_ps", bufs=4, space=bass.MemorySpace.PSUM) as score_pp,
                tc.tile_pool(name="wt_ps", bufs=2, space=bass.MemorySpace.PSUM) as wt_pp,
                tc.tile_pool(name="out_ps", bufs=2, space=bass.MemorySpace.PSUM) as out_pp,
            ):
                n_groups = N_LOC // 128
                for g in range(n_groups):
                    sc = [
                        score_pp.tile([128, 512], F32, tag="sc", name=f"sc{g}_{jh}")
                        for jh in range(2)
                    ]
                    ig = 0
                    for bi, ib in enumerate(_blocks(g == 0, g == n_groups - 1)):
                        S = s_pool.tile([128, IB, M], F32, tag="S", name=f"S{g}_{bi}")
                        if g == 0 and bi == 0:
                            for jh in range(2):
                                js = slice(jh * 512, (jh + 1) * 512)
                                for il in range(ib):
                                    i = g * 128 + ig + il
                                    nc.vector.tensor_scalar_add(
                                        S[:, il, js], tkT_sb[:, js], tqT_sb[:, i : i + 1]
                                    )
                        else:
                            for il in range(ib):
                                i = g * 128 + ig + il
                                nc.vector.tensor_scalar_add(
                                    S[:, il, :], tkT_sb[:, :], tqT_sb[:, i : i + 1]
                                )
                        T = t_pool.tile([128, IB, M], BF16, tag="T", name=f"T{g}_{bi}")
                        nc.scalar.activation(T[:, :ib, :], S[:, :ib, :], tanh)
                        if g == 0 and bi == 0:
                            # non-critical loads, issued once the hot path rolls
                            nc.sync.dma_start(
                                v_bf[:, :, :], v_d.rearrange("(t p) v -> p t v", p=128)
                            )
                            nc.sync.dma_start(
                                badd_sb[:, :, :],
                                badd_d.rearrange("(t p) j -> p t j", p=128),
                            )
                        for il in range(ib):
                            r = ig + il
                            for jh in range(2):
                                nc.tensor.matmul(
                                    sc[jh][:, :],
                                    msl[:, H - r : 2 * H - r],
                                    T[:, il, jh * 512 : (jh + 1) * 512],
                                    start=(r == 0), stop=(r == 127),
                                )
                        ig += ib

                    # ---- softmax + output for this 128-row group ----
                    scm = sm_pool.tile([128, M], F32)
                    for jh in range(2):
                        nc.vector.tensor_add(
                            scm[:, jh * 512 : (jh + 1) * 512],
                            sc[jh][:, :],
                            badd_sb[:, g, jh * 512 : (jh + 1) * 512],
                        )
                    wexp = w_pool.tile([128, M], BF16)
                    rowsum = stat.tile([128, 1], F32)
                    nc.scalar.activation(
                        wexp[:, :], scm[:, :], expf, accum_out=rowsum[:, 0:1]
                    )
                    recip = stat.tile([128, 1], F32)
                    nc.vector.reciprocal(recip[:, 0:1], rowsum[:, 0:1])

                    wt_sb = wt_pool.tile([128, M // 128, 128], BF16)
                    for jt in range(M // 128):
                        wt_ps = wt_pp.tile([128, 128], BF16)
                        nc.tensor.transpose(
                            wt_ps[:, :], wexp[:, jt * 128 : (jt + 1) * 128], ident[:, :]
                        )
                        nc.vector.tensor_copy(wt_sb[:, jt, :], wt_ps[:, :])

                    out_ps = out_pp.tile([128, DV], F32)
                    for jt in range(M // 128):
                        nc.tensor.matmul(
                            out_ps[:, :], wt_sb[:, jt, :], v_bf[:, jt, :],
                            start=(jt == 0), stop=(jt == M // 128 - 1),
                        )
                    out_sb = o_pool.tile([128, DV], F32)
                    nc.vector.tensor_scalar_mul(out_sb[:, :], out_ps[:, :], recip[:, 0:1])
                    nc.sync.dma_start(out_d[g * 128 : (g + 1) * 128, :], out_sb[:, :])

    nc.compile()
    return nc


_NC_CACHE = []


def _get_nc():
    if not _NC_CACHE:
        _NC_CACHE.append(build_nc())
    return _NC_CACHE[0]


def make_in_maps(queries, keys, values, mask, Wq, bq, Wk, bk, wv, bv):
    f32 = np.float32
    bf = ml_dtypes.bfloat16
    badd_full = (mask.astype(f32) - 1.0) * -NEG  # 0 where valid, NEG where masked
    wv_col = np.ascontiguousarray(wv.reshape(H, 1).astype(f32))
    bqk = np.ascontiguousarray((bq + bk).reshape(H, 1).astype(f32))
    wq = np.ascontiguousarray(Wq.astype(bf))
    wk = np.ascontiguousarray(Wk.astype(bf))
    in_maps = []
    for c in range(N_CORES):
        b, half = divmod(c, 2)
        rows = slice(half * N_LOC, (half + 1) * N_LOC)
        in_maps.append(
            {
                "qT": np.ascontiguousarray(queries[b, rows].T.astype(bf)),
                "kT": np.ascontiguousarray(keys[b].T.astype(bf)),
                "v": np.ascontiguousarray(values[b].astype(bf)),
                "badd": np.ascontiguousarray(badd_full[b, rows]),
                "Wq": wq,
                "Wk": wk,
                "wv": wv_col,
                "bqk": bqk,
            }
        )
    return in_maps


def gather_out(results):
    out = np.zeros((B, N, DV), np.float32)
    for c in range(N_CORES):
        b, half = divmod(c, 2)
        out[b, half * N_LOC : (half + 1) * N_LOC] = results[c]["out"]
    return out


def kernel(**inputs):
    nc = _get_nc()
    in_maps = make_in_maps(**inputs)
    res = run_bass_kernel_spmd(nc, in_maps, core_ids=list(range(N_CORES)))
    return gather_out(res.results)
